# revision 5
# baseline (speedup 1.0000x reference)
"""Mega (Flash-Quad) encoder layer on 8 Trainium2 NeuronCores.

Sharding: data-parallel over batch B=16 -> Bc=2 per core. Everything inside a
core: MultiHeadEMA as 4 first-order DVE scans per channel block (2 EMA dims x
2 directions, decay/weight scalars precomputed on host), projections as
fp32r PE matmuls (token-major outputs, feature-major z), chunked attention
with fp16 score/AV matmuls, gated residual + ScaleNorm fused per 128-token
chunk.
"""

import numpy as np

L, B, D = 2048, 16, 512
H, Z, NDIM = 1024, 128, 2
CHUNK = 128
MAXPOS = 1024
EPS = 1e-6
N_CORES = 8
BC = B // N_CORES          # batches per core
NCH = L // CHUNK           # chunks per batch
DO = D // 128              # d_outer tiles

_PROG_CACHE = {}


def _build_program():
    if "nc" in _PROG_CACHE:
        return _PROG_CACHE["nc"]

    import concourse.bass as bass
    import concourse.mybir as mybir
    import concourse.tile as tile
    from concourse import bacc
    from concourse.masks import make_identity

    f32 = mybir.dt.float32
    f32r = mybir.dt.float32r
    f16 = mybir.dt.float16
    AF = mybir.ActivationFunctionType
    OP = mybir.AluOpType
    AX = mybir.AxisListType

    nc = bacc.Bacc(target_bir_lowering=False)

    x_in = nc.declare_dram_parameter("x", [L, BC, D], f32, isOutput=False)
    Wv_in = nc.declare_dram_parameter("Wv", [D, H], f32, isOutput=False)
    Wmx_in = nc.declare_dram_parameter("Wmx", [D, 2 * D + H + Z], f32, isOutput=False)
    Wh_in = nc.declare_dram_parameter("Wh", [H, D], f32, isOutput=False)
    emaq_in = nc.declare_dram_parameter("ema_q", [DO, 128, 4], f32, isOutput=False)
    emaw_in = nc.declare_dram_parameter("ema_w", [DO, 128, 4], f32, isOutput=False)
    omega_in = nc.declare_dram_parameter("omega", [DO, 128, 1], f32, isOutput=False)
    qkaff_in = nc.declare_dram_parameter("qk_aff", [128, 4], f32, isOutput=False)
    bv_in = nc.declare_dram_parameter("bv_rep", [128, H], f32, isOutput=False)
    bu_in = nc.declare_dram_parameter("bu_rep", [128, D], f32, isOutput=False)
    br_in = nc.declare_dram_parameter("br_rep", [128, H], f32, isOutput=False)
    bhx_in = nc.declare_dram_parameter("bhx_rep", [128, D], f32, isOutput=False)
    bz_in = nc.declare_dram_parameter("bz", [128, 1], f32, isOutput=False)
    btoep_in = nc.declare_dram_parameter("bias_toep", [128, 128], f32, isOutput=False)
    ns_in = nc.declare_dram_parameter("ns", [128, 1], f32, isOutput=False)

    out_p = nc.declare_dram_parameter("out", [L, BC, D], f32, isOutput=True)
    attn_p = nc.declare_dram_parameter("attn", [BC, NCH, CHUNK, CHUNK], f32, isOutput=True)

    # DRAM scratch: feature-major x^T and mx  (dout, d_in, b, t)
    xT_s = nc.dram_tensor("xT_s", [DO, 128, BC, L], f32)
    mx_s = nc.dram_tensor("mx_s", [DO, 128, BC, L], f32)

    # column layout of the reordered Wmx (host permutes): [u(512) hx(512) r(1024) z(128)]
    CU, CHX, CR, CZ = 0, D, 2 * D, 2 * D + H

    with tile.TileContext(nc) as tc:
        import contextlib
        ctx = contextlib.ExitStack()
        const = ctx.enter_context(tc.tile_pool(name="const", bufs=1))

        # ---- resident constants ----
        Wv_sb = const.tile([128, DO, H], f32r)
        nc.sync.dma_start(Wv_sb[:], Wv_in.ap().rearrange("(o p) h -> p o h", p=128).bitcast(f32r))
        Wmx_sb = const.tile([128, DO, 2 * D + H + Z], f32r)
        nc.sync.dma_start(Wmx_sb[:], Wmx_in.ap().rearrange("(o p) h -> p o h", p=128).bitcast(f32r))
        Wh_sb = const.tile([128, H // 128, D], f32r)
        nc.sync.dma_start(Wh_sb[:], Wh_in.ap().rearrange("(o p) h -> p o h", p=128).bitcast(f32r))
        emaq_sb = const.tile([128, DO, 4], f32)
        nc.sync.dma_start(emaq_sb[:], emaq_in.ap().rearrange("o p j -> p o j"))
        emaw_sb = const.tile([128, DO, 4], f32)
        nc.sync.dma_start(emaw_sb[:], emaw_in.ap().rearrange("o p j -> p o j"))
        omega_sb = const.tile([128, DO, 1], f32)
        nc.sync.dma_start(omega_sb[:], omega_in.ap().rearrange("o p j -> p o j"))
        qkaff_sb = const.tile([128, 4], f32)
        nc.sync.dma_start(qkaff_sb[:], qkaff_in.ap())
        bv_sb = const.tile([128, H], f32)
        nc.sync.dma_start(bv_sb[:], bv_in.ap())
        bu_sb = const.tile([128, D], f32)
        nc.sync.dma_start(bu_sb[:], bu_in.ap())
        br_sb = const.tile([128, H], f32)
        nc.sync.dma_start(br_sb[:], br_in.ap())
        bhx_sb = const.tile([128, D], f32)
        nc.sync.dma_start(bhx_sb[:], bhx_in.ap())
        bz_sb = const.tile([128, 1], f32)
        nc.sync.dma_start(bz_sb[:], bz_in.ap())
        btoep_sb = const.tile([128, 128], f32)
        nc.sync.dma_start(btoep_sb[:], btoep_in.ap())
        ns_sb = const.tile([128, 1], f32)
        nc.sync.dma_start(ns_sb[:], ns_in.ap())
        ident = const.tile([128, 128], f32)
        make_identity(nc, ident[:])
        # feature-major shared z (fp16), resident across phases
        z_sb = const.tile([128, BC, L], f16)

        # ================= Phase A: transpose x + EMA scans =================
        with tc.tile_pool(name="pa_big", bufs=2) as big, \
             tc.tile_pool(name="pa_tmp", bufs=1) as tmp, \
             tc.tile_pool(name="pa_h", bufs=1) as hpool, \
             tc.tile_pool(name="pa_sm", bufs=3) as sm, \
             tc.tile_pool(name="pa_ps", bufs=3, space="PSUM") as ps128a:
            for b in range(BC):
                for do in range(DO):
                    xT_sl = big.tile([128, L], f32, tag="xT_sl")
                    for lt in range(L // 128):
                        xt = sm.tile([128, 128], f32, tag="xt")
                        nc.sync.dma_start(xt[:], x_in.ap()[lt * 128:(lt + 1) * 128, b, do * 128:(do + 1) * 128])
                        pst = ps128a.tile([128, 128], f32, tag="pst")
                        nc.tensor.transpose(pst[:], xt[:], ident[:])
                        nc.scalar.activation(xT_sl[:, lt * 128:(lt + 1) * 128], pst[:], AF.Copy)
                    nc.sync.dma_start(xT_s.ap()[do, :, b, :], xT_sl[:])

                    h4 = hpool.tile([128, 4, L], f32, tag="h4")
                    for j in range(4):
                        xw = big.tile([128, L], f32, tag="xw")
                        nc.scalar.activation(xw[:], xT_sl[:], AF.Copy,
                                             scale=emaw_sb[:, do, j:j + 1])
                        qb = emaq_sb[:, do, j:j + 1].to_broadcast((128, L))
                        if j < 2:
                            nc.vector.tensor_tensor_scan(h4[:, j], qb, xw[:],
                                                         0.0, OP.mult, OP.add)
                        else:
                            nc.vector.tensor_tensor_scan(h4[:, j, ::-1], qb, xw[:, ::-1],
                                                         0.0, OP.mult, OP.add)
                    t1 = tmp.tile([128, L], f32, tag="t1")
                    nc.vector.tensor_tensor(t1[:], h4[:, 0], h4[:, 1], OP.add)
                    t2 = tmp.tile([128, L], f32, tag="t2")
                    nc.gpsimd.tensor_tensor(t2[:], h4[:, 2], h4[:, 3], OP.add)
                    ox = tmp.tile([128, L], f32, tag="ox")
                    nc.scalar.activation(ox[:], xT_sl[:], AF.Copy,
                                         scale=omega_sb[:, do, 0:1])
                    nc.vector.tensor_tensor(t1[:], t1[:], t2[:], OP.add)
                    nc.gpsimd.tensor_tensor(t1[:], t1[:], ox[:], OP.add)
                    mx_sl = big.tile([128, L], f32, tag="mx_sl")
                    nc.scalar.activation(mx_sl[:], t1[:], AF.Silu)
                    nc.sync.dma_start(mx_s.ap()[do, :, b, :], mx_sl[:])

        # ================= Phase B: z projection (feature-major, batched) =====
        with tc.tile_pool(name="pb", bufs=2) as pb, \
             tc.tile_pool(name="pb_ps", bufs=2, space="PSUM") as ps512b:
            for b in range(BC):
                for ts in range(L // 512):
                    mxz = pb.tile([128, DO, 512], f32r, tag="mxz")
                    nc.sync.dma_start(
                        mxz[:], mx_s.ap()[:, :, b, ts * 512:(ts + 1) * 512]
                        .rearrange("o p t -> p o t").bitcast(f32r))
                    psz = ps512b.tile([128, 512], f32, tag="psz")
                    for do in range(DO):
                        nc.tensor.matmul(psz[:], Wmx_sb[:, do, CZ:CZ + Z], mxz[:, do],
                                         start=(do == 0), stop=(do == DO - 1))
                    nc.scalar.activation(z_sb[:, b, ts * 512:(ts + 1) * 512], psz[:],
                                         AF.Silu, bias=bz_sb[:, 0:1])

        # ================= Phase C: per-chunk stream =================
        ck = ctx.enter_context(tc.tile_pool(name="ck", bufs=2))
        ps512 = ctx.enter_context(tc.tile_pool(name="ps512", bufs=4, space="PSUM"))
        ps128 = ctx.enter_context(tc.tile_pool(name="ps128", bufs=4, space="PSUM"))
        for tidx in range(BC * NCH):
            b, c = tidx // NCH, tidx % NCH
            tok = slice(c * 128, (c + 1) * 128)

            xT_c = ck.tile([128, DO, 128], f32r, tag="xT_c")
            nc.sync.dma_start(xT_c[:], xT_s.ap()[:, :, b, tok]
                              .rearrange("o p t -> p o t").bitcast(f32r))
            mx_c = ck.tile([128, DO, 128], f32r, tag="mx_c")
            nc.sync.dma_start(mx_c[:], mx_s.ap()[:, :, b, tok]
                              .rearrange("o p t -> p o t").bitcast(f32r))

            # ---- v = silu(x @ Wv + bv)  (token-major, fp16) ----
            vtmp = ck.tile([128, H], f32, tag="tmp1024")
            for half in range(2):
                psv = ps512.tile([128, 512], f32, tag="mm512")
                for do in range(DO):
                    nc.tensor.matmul(psv[:], xT_c[:, do],
                                     Wv_sb[:, do, half * 512:(half + 1) * 512],
                                     start=(do == 0), stop=(do == DO - 1))
                nc.vector.tensor_tensor(vtmp[:, half * 512:(half + 1) * 512], psv[:],
                                        bv_sb[:, half * 512:(half + 1) * 512], OP.add)
            v_c = ck.tile([128, H], f16, tag="v_c")
            nc.scalar.activation(v_c[:], vtmp[:], AF.Silu)

            # ---- u, hx (token-major) ----
            psu = ps512.tile([128, 512], f32, tag="mm512")
            pshx = ps512.tile([128, 512], f32, tag="mm512")
            for do in range(DO):
                nc.tensor.matmul(psu[:], mx_c[:, do], Wmx_sb[:, do, CU:CU + D],
                                 start=(do == 0), stop=(do == DO - 1))
            for do in range(DO):
                nc.tensor.matmul(pshx[:], mx_c[:, do], Wmx_sb[:, do, CHX:CHX + D],
                                 start=(do == 0), stop=(do == DO - 1))
            upre = ck.tile([128, D], f32, tag="tmp512")
            nc.vector.tensor_tensor(upre[:], psu[:], bu_sb[:], OP.add)
            u_c = ck.tile([128, D], f32, tag="u_c")
            nc.scalar.activation(u_c[:], upre[:], AF.Sigmoid)
            hx_c = ck.tile([128, D], f32, tag="hx_c")
            nc.vector.tensor_tensor(hx_c[:], pshx[:], bhx_sb[:], OP.add)

            # ---- r = silu(...) (token-major) ----
            rtmp = ck.tile([128, H], f32, tag="tmp1024")
            for half in range(2):
                psr = ps512.tile([128, 512], f32, tag="mm512")
                for do in range(DO):
                    nc.tensor.matmul(psr[:], mx_c[:, do],
                                     Wmx_sb[:, do, CR + half * 512:CR + (half + 1) * 512],
                                     start=(do == 0), stop=(do == DO - 1))
                nc.vector.tensor_tensor(rtmp[:, half * 512:(half + 1) * 512], psr[:],
                                        br_sb[:, half * 512:(half + 1) * 512], OP.add)
            r_c = ck.tile([128, H], f32, tag="r_c")
            nc.scalar.activation(r_c[:], rtmp[:], AF.Silu)

            # ---- attention scores ----
            q_c = ck.tile([128, 128], f16, tag="q_c")
            nc.vector.tensor_scalar(q_c[:], z_sb[:, b, tok], qkaff_sb[:, 0:1],
                                    qkaff_sb[:, 1:2], OP.mult, OP.add)
            k_c = ck.tile([128, 128], f16, tag="k_c")
            nc.vector.tensor_scalar(k_c[:], z_sb[:, b, tok], qkaff_sb[:, 2:3],
                                    qkaff_sb[:, 3:4], OP.mult, OP.add)
            pss = ps128.tile([128, 128], f32, tag="mm128")
            nc.tensor.matmul(pss[:], q_c[:], k_c[:], start=True, stop=True)
            S_sb = ck.tile([128, 128], f32, tag="S_sb")
            nc.vector.tensor_tensor(S_sb[:], pss[:], btoep_sb[:], OP.add)
            mneg = ck.tile([128, 1], f32, tag="mneg")
            nc.vector.tensor_reduce(mneg[:], S_sb[:], AX.X, OP.max, negate=True)
            E_sb = ck.tile([128, 128], f32, tag="E_sb")
            esum = ck.tile([128, 1], f32, tag="esum")
            nc.scalar.activation(E_sb[:], S_sb[:], AF.Exp, bias=mneg[:, 0:1],
                                 accum_out=esum[:, 0:1])
            rs = ck.tile([128, 1], f32, tag="rs")
            nc.vector.reciprocal(rs[:], esum[:])
            A_sb = ck.tile([128, 128], f32, tag="A_sb")
            nc.vector.tensor_scalar(A_sb[:], E_sb[:], rs[:, 0:1], None, OP.mult)
            nc.sync.dma_start(attn_p.ap()[b, c], A_sb[:])

            psat = ps128.tile([128, 128], f32, tag="mm128")
            nc.tensor.transpose(psat[:], A_sb[:], ident[:])
            at_c = ck.tile([128, 128], f16, tag="at_c")
            nc.scalar.activation(at_c[:], psat[:], AF.Copy)

            # ---- h_attn = A @ v ; rh = h_attn * r ----
            rh = ck.tile([128, H], f32, tag="rh")
            for half in range(2):
                psh = ps512.tile([128, 512], f32, tag="mm512")
                nc.tensor.matmul(psh[:], at_c[:], v_c[:, half * 512:(half + 1) * 512],
                                 start=True, stop=True)
                nc.vector.tensor_tensor(rh[:, half * 512:(half + 1) * 512], psh[:],
                                        r_c[:, half * 512:(half + 1) * 512], OP.mult)

            # ---- h = silu(hx + rh @ Wh) ----
            rhT = ck.tile([128, H // 128, 128], f32r, tag="rhT")
            for hh in range(H // 128):
                psrt = ps128.tile([128, 128], f32, tag="mm128")
                nc.tensor.transpose(psrt[:], rh[:, hh * 128:(hh + 1) * 128], ident[:])
                nc.scalar.activation(rhT[:, hh], psrt[:], AF.Copy)
            pso = ps512.tile([128, 512], f32, tag="mm512")
            for hh in range(H // 128):
                nc.tensor.matmul(pso[:], rhT[:, hh], Wh_sb[:, hh, :],
                                 start=(hh == 0), stop=(hh == H // 128 - 1))
            hpre = ck.tile([128, D], f32, tag="tmp512")
            nc.vector.tensor_tensor(hpre[:], pso[:], hx_c[:], OP.add)
            h_c = ck.tile([128, D], f32, tag="h_c")
            nc.scalar.activation(h_c[:], hpre[:], AF.Silu)

            # ---- gated residual + ScaleNorm ----
            x_c = ck.tile([128, D], f32, tag="x_c")
            nc.sync.dma_start(x_c[:], x_in.ap()[tok, b, :])
            d_c = ck.tile([128, D], f32, tag="d_c")
            nc.gpsimd.tensor_tensor(d_c[:], h_c[:], x_c[:], OP.subtract)
            g_c = ck.tile([128, D], f32, tag="g_c")
            nc.vector.tensor_tensor(g_c[:], u_c[:], d_c[:], OP.mult)
            o_c = ck.tile([128, D], f32, tag="o_c")
            nc.gpsimd.tensor_tensor(o_c[:], x_c[:], g_c[:], OP.add)

            sqt = ck.tile([128, D], f32, tag="tmp512")
            ssq = ck.tile([128, 1], f32, tag="ssq")
            nc.scalar.activation(sqt[:], o_c[:], AF.Square, accum_out=ssq[:, 0:1])
            ms = ck.tile([128, 1], f32, tag="ms")
            nc.vector.tensor_scalar(ms[:], ssq[:], 1.0 / D, EPS, OP.mult, OP.add)
            sq2 = ck.tile([128, 1], f32, tag="sq2")
            nc.scalar.activation(sq2[:], ms[:], AF.Sqrt)
            rinv = ck.tile([128, 1], f32, tag="rinv")
            nc.vector.reciprocal(rinv[:], sq2[:])
            scl = ck.tile([128, 1], f32, tag="scl")
            nc.vector.tensor_tensor(scl[:], rinv[:], ns_sb[:], OP.mult)
            outt = ck.tile([128, D], f32, tag="outt")
            nc.vector.tensor_scalar(outt[:], o_c[:], scl[:, 0:1], None, OP.mult)
            nc.sync.dma_start(out_p.ap()[tok, b, :], outt[:])

        ctx.close()

    nc.finalize()
    _PROG_CACHE["nc"] = nc
    return nc


def _sigmoid(x):
    return 1.0 / (1.0 + np.exp(-x))


def kernel(**inputs):
    from concourse.bass_utils import run_bass_kernel_spmd

    nc = _build_program()

    x = np.asarray(inputs["x"], dtype=np.float32)
    Wv = np.asarray(inputs["Wv"], dtype=np.float32)
    bv = np.asarray(inputs["bv"], dtype=np.float32)
    Wmx = np.asarray(inputs["Wmx"], dtype=np.float32)
    bmx = np.asarray(inputs["bmx"], dtype=np.float32)
    Wh = np.asarray(inputs["Wh"], dtype=np.float32)
    bh = np.asarray(inputs["bh"], dtype=np.float32)
    qk_gamma = np.asarray(inputs["qk_gamma"], dtype=np.float64)
    qk_beta = np.asarray(inputs["qk_beta"], dtype=np.float64)
    rel = np.asarray(inputs["rel_pos_bias"], dtype=np.float32)
    ns = np.float32(np.asarray(inputs["norm_scalar"], dtype=np.float32))

    delta = np.asarray(inputs["ema_delta"], dtype=np.float64)[:, :, 0]   # (2D, n)
    alpha = np.asarray(inputs["ema_alpha"], dtype=np.float64)[:, :, 0]
    beta = np.asarray(inputs["ema_beta"], dtype=np.float64)[:, :, 0]
    gamma = np.asarray(inputs["ema_gamma"], dtype=np.float64)            # (2D, n)
    omega = np.asarray(inputs["ema_omega"], dtype=np.float32)            # (D,)

    p = _sigmoid(delta)
    q = 1.0 - p * _sigmoid(alpha)                                        # (2D, n)
    w = p * beta * gamma / np.sqrt(NDIM)                                 # (2D, n)

    # layout (DO, 128, 4): j = [causal n0, causal n1, anti n0, anti n1]
    qc, qa = q[:D], q[D:]
    wc, wa = w[:D], w[D:]
    ema_q = np.concatenate([qc, qa], axis=1).reshape(DO, 128, 4).astype(np.float32)
    ema_w = np.concatenate([wc, wa], axis=1).reshape(DO, 128, 4).astype(np.float32)
    omega_a = omega.reshape(DO, 128, 1).astype(np.float32)

    # Wmx column reorder: [u(D) hx(D) r(H) z(Z)] from original [u(D) z(Z) r(H) hx(D)]
    perm = np.concatenate([
        np.arange(0, D),                      # u
        np.arange(D + Z + H, 2 * D + Z + H),  # hx
        np.arange(D + Z, D + Z + H),          # r
        np.arange(D, D + Z),                  # z
    ])
    Wmx_p = np.ascontiguousarray(Wmx[:, perm])
    bmx_p = bmx[perm]

    ones = np.ones((128, 1), dtype=np.float32)
    bv_rep = np.ascontiguousarray(ones * bv[None, :])
    bu_rep = np.ascontiguousarray(ones * bmx_p[None, :D])
    bhx_rep = np.ascontiguousarray(ones * (bmx_p[D:2 * D] + bh)[None, :])
    br_rep = np.ascontiguousarray(ones * bmx_p[None, 2 * D:2 * D + H])
    bz = np.ascontiguousarray(bmx_p[2 * D + H:, None])                   # (Z=128, 1)

    s = Z ** -0.5
    qk_aff = np.stack([qk_gamma[0] * s, qk_beta[0] * s,
                       qk_gamma[1], qk_beta[1]], axis=1).astype(np.float32)  # (128, 4)

    idx = (MAXPOS - 1) + np.arange(CHUNK)[None, :] - np.arange(CHUNK)[:, None]
    bias_toep = np.ascontiguousarray(rel[idx]).astype(np.float32)

    ns_rep = np.full((128, 1), ns, dtype=np.float32)

    shared = dict(Wv=Wv, Wmx=Wmx_p, Wh=Wh, ema_q=ema_q, ema_w=ema_w,
                  omega=omega_a, qk_aff=qk_aff, bv_rep=bv_rep, bu_rep=bu_rep,
                  br_rep=br_rep, bhx_rep=bhx_rep, bz=bz, bias_toep=bias_toep,
                  ns=ns_rep)
    in_maps = []
    for k in range(N_CORES):
        m = dict(shared)
        m["x"] = np.ascontiguousarray(x[:, k * BC:(k + 1) * BC, :])
        in_maps.append(m)

    res = run_bass_kernel_spmd(nc, in_maps, core_ids=list(range(N_CORES)))

    out = np.empty((L, B, D), dtype=np.float32)
    attn = np.empty((B, NCH, CHUNK, CHUNK), dtype=np.float32)
    for k in range(N_CORES):
        out[:, k * BC:(k + 1) * BC, :] = res.results[k]["out"]
        attn[k * BC:(k + 1) * BC] = res.results[k]["attn"]
    return out, attn
